# revision 5
# baseline (speedup 1.0000x reference)
"""Trainium2 Bass kernel for CounterfactualAnswerLoss.

Math notes (verified against the reference):
  - The random permutation (argsort of keyed noise) maps the k active slots
    onto themselves, and the result is immediately summed over the slot axis
    by the einsum 'bkv,vd->bd'.  The permutation therefore cancels: only
    s[b,:] = sum_{j<k_b} row_j matters, where row_j is p_z[b,j] when the
    permute branch is taken and mix_samples[b,j]/max(sum_v, eps) otherwise.
  - use_perm[b] = (coin_u[b] < 0.5) & (k_vals[b] >= 2).
  - digit_logits_cf = (s @ W) / K;  then softmax + JS divergence vs
    softmax(digit_logits_ref), meaned over B, negated.

Strategy: the host picks, per batch element, which source tensor the device
needs (only tiny metadata: k_vals/coin_u), load-balances batch elements
across the 8 cores by row count (LPT), and ships only the needed rows in a
partition-major layout.  The device does all heavy work: reads every
selected row once, projects it through [W | ones] (fp32 matmuls on the
tensor engine, accumulating over the 250 contraction chunks), normalizes
mix rows by their row sum, reduces rows into per-batch logits with a
per-core assignment matmul, and computes softmax + JS on-device.  Host sums
the 8 per-core JS partial vectors (the "all-reduce") and scales.
"""

import numpy as np

P = 128          # SBUF partitions / contraction tile
V = 32000        # vocab
IV = V // P      # 250 inner elements per partition
D = 10           # digits
DD = D + 1       # W columns + ones column
KMAX = 16
N_CORES = 8
R_TILE = 64      # rows per SBUF tile
EPS = 1e-8

_prog_cache: dict = {}


def _build_program(n_slots: int, n_b: int):
    """Build the SPMD Bass/Tile program for one core shape."""
    from contextlib import ExitStack

    import concourse.bacc as bacc
    import concourse.mybir as mybir
    import concourse.tile as tile

    f32 = mybir.dt.float32
    AF = mybir.ActivationFunctionType
    ALU = mybir.AluOpType
    AX = mybir.AxisListType

    nc = bacc.Bacc(
        "TRN2", target_bir_lowering=False, debug=False, num_devices=N_CORES
    )
    data = nc.dram_tensor("data", [P, n_slots, IV], f32, kind="ExternalInput").ap()
    w1 = nc.dram_tensor("w1", [P, IV, DD], f32, kind="ExternalInput").ap()
    amat = nc.dram_tensor("amat", [n_slots, n_b + 2], f32, kind="ExternalInput").ap()
    lref = nc.dram_tensor("lref", [n_b, D], f32, kind="ExternalInput").ap()
    jsout = nc.dram_tensor("jsout", [n_b, 1], f32, kind="ExternalOutput").ap()

    tiles = []
    r0 = 0
    while r0 < n_slots:
        r = min(R_TILE, n_slots - r0)
        tiles.append((r0, r))
        r0 += r

    with tile.TileContext(nc) as tc, ExitStack() as ctx:
        const_pool = ctx.enter_context(tc.tile_pool(name="const", bufs=1))
        data_pool = ctx.enter_context(tc.tile_pool(name="data", bufs=2))
        am_pool = ctx.enter_context(tc.tile_pool(name="am", bufs=2))
        small = ctx.enter_context(tc.tile_pool(name="small", bufs=2))
        ep = ctx.enter_context(tc.tile_pool(name="ep", bufs=1))
        ypool = ctx.enter_context(tc.tile_pool(name="y", bufs=2, space="PSUM"))
        lgpool = ctx.enter_context(tc.tile_pool(name="lg", bufs=1, space="PSUM"))

        w1_sb = const_pool.tile([P, IV, DD], f32)
        nc.sync.dma_start(w1_sb[:], w1[:])
        lref_sb = const_pool.tile([n_b, D], f32)
        nc.sync.dma_start(lref_sb[:], lref[:])

        lg = lgpool.tile([n_b, D], f32)
        n_t = len(tiles)
        for t, (r0, r) in enumerate(tiles):
            dt_sb = data_pool.tile([P, R_TILE, IV], f32, tag="dt")
            nc.sync.dma_start(dt_sb[:, :r, :], data[:, r0 : r0 + r, :])
            am = am_pool.tile([R_TILE, n_b + 2], f32, tag="am")
            nc.sync.dma_start(am[:r, :], amat[r0 : r0 + r, :])

            # y[r, 0:10] = row @ W ; y[r, 10] = row sum — accumulated over
            # the 250 partition-chunks of the V contraction.
            y = ypool.tile([R_TILE, DD], f32, tag="y")
            for i in range(IV):
                nc.tensor.matmul(
                    y[:r, :],
                    dt_sb[:, :r, i],
                    w1_sb[:, i, :],
                    start=(i == 0),
                    stop=(i == IV - 1),
                )

            # c = 1 for plain rows, 1/max(rowsum, eps) for mix rows:
            # c = rcp*mixsel - (mixsel - 1)
            rsm = small.tile([R_TILE, 1], f32, tag="rsm")
            nc.vector.tensor_scalar_max(rsm[:r, :], y[:r, D:DD], EPS)
            rcp = small.tile([R_TILE, 1], f32, tag="rcp")
            nc.vector.reciprocal(rcp[:r, :], rsm[:r, :])
            c = small.tile([R_TILE, 1], f32, tag="c")
            nc.vector.scalar_tensor_tensor(
                c[:r, :],
                rcp[:r, :],
                am[:r, n_b : n_b + 1],
                am[:r, n_b + 1 : n_b + 2],
                op0=ALU.mult,
                op1=ALU.subtract,
            )
            ysc = small.tile([R_TILE, D], f32, tag="ysc")
            nc.vector.tensor_scalar_mul(ysc[:r, :], y[:r, 0:D], c[:r, 0:1])

            # logits[b_slot, d] += sum_r A[r, b_slot] * ysc[r, d]
            # (A entries carry the 1/K factor.)
            nc.tensor.matmul(
                lg[:, :],
                am[:r, 0:n_b],
                ysc[:r, :],
                start=(t == 0),
                stop=(t == n_t - 1),
            )

        # ---- epilogue: softmax both, then JS ----
        def softmax_clipped(src_ap, tag):
            mx = ep.tile([n_b, 1], f32, tag=f"mx{tag}", name=f"mx{tag}")
            nc.vector.tensor_reduce(mx[:], src_ap, axis=AX.X, op=ALU.max)
            nmx = ep.tile([n_b, 1], f32, tag=f"nmx{tag}", name=f"nmx{tag}")
            nc.vector.tensor_scalar_mul(nmx[:], mx[:], -1.0)
            ex = ep.tile([n_b, D], f32, tag=f"ex{tag}", name=f"ex{tag}")
            den = ep.tile([n_b, 1], f32, tag=f"den{tag}", name=f"den{tag}")
            nc.scalar.activation(
                ex[:], src_ap, AF.Exp, bias=nmx[:, 0:1], scale=1.0, accum_out=den[:]
            )
            rden = ep.tile([n_b, 1], f32, tag=f"rden{tag}", name=f"rden{tag}")
            nc.vector.reciprocal(rden[:], den[:])
            prob = ep.tile([n_b, D], f32, tag=f"prob{tag}", name=f"prob{tag}")
            nc.vector.tensor_scalar(
                prob[:], ex[:], rden[:, 0:1], EPS, op0=ALU.mult, op1=ALU.max
            )
            return prob

        q = softmax_clipped(lg[:, :], "q")       # p_cf
        p = softmax_clipped(lref_sb[:], "p")     # p_ref

        pq = ep.tile([n_b, D], f32)
        nc.vector.tensor_add(pq[:], p[:], q[:])
        lm = ep.tile([n_b, D], f32)
        nc.scalar.activation(lm[:], pq[:], AF.Ln, scale=0.5)
        lp = ep.tile([n_b, D], f32)
        nc.scalar.activation(lp[:], p[:], AF.Ln)
        lq = ep.tile([n_b, D], f32)
        nc.scalar.activation(lq[:], q[:], AF.Ln)

        d1 = ep.tile([n_b, D], f32)
        nc.vector.tensor_sub(d1[:], lp[:], lm[:])
        t1 = ep.tile([n_b, D], f32)
        nc.vector.tensor_mul(t1[:], p[:], d1[:])
        kl1 = ep.tile([n_b, 1], f32)
        nc.vector.tensor_reduce(kl1[:], t1[:], axis=AX.X, op=ALU.add)

        d2 = ep.tile([n_b, D], f32)
        nc.vector.tensor_sub(d2[:], lq[:], lm[:])
        t2 = ep.tile([n_b, D], f32)
        nc.vector.tensor_mul(t2[:], q[:], d2[:])
        kl2 = ep.tile([n_b, 1], f32)
        nc.vector.tensor_reduce(kl2[:], t2[:], axis=AX.X, op=ALU.add)

        kls = ep.tile([n_b, 1], f32)
        nc.vector.tensor_add(kls[:], kl1[:], kl2[:])
        nc.sync.dma_start(jsout[:], kls[:])

    nc.compile()
    return nc


def _prepare(inputs):
    """Host-side selection + sharding. Returns (n_slots, n_b, in_maps, counts)."""
    p_z = np.asarray(inputs["p_z"])
    dlr = np.asarray(inputs["digit_logits_ref"], dtype=np.float32)
    k_vals = np.asarray(inputs["k_vals"]).astype(np.int64)
    coin_u = np.asarray(inputs["coin_u"], dtype=np.float32)
    mix = np.asarray(inputs["mix_samples"])
    W = np.asarray(inputs["W"], dtype=np.float32)
    B, K, Vv = p_z.shape
    assert (K, Vv) == (KMAX, V) and B == 128

    kprob = np.where(k_vals >= 2, np.float32(0.5), np.float32(0.0))
    use_perm = (coin_u < kprob) & (k_vals > 1)

    # LPT assignment of batch elements to cores, balancing row counts.
    order = np.argsort(-k_vals, kind="stable")
    loads = [0] * N_CORES
    assign: list[list[int]] = [[] for _ in range(N_CORES)]
    for b in order:
        c = min(range(N_CORES), key=lambda cc: (loads[cc], len(assign[cc])))
        assign[c].append(int(b))
        loads[c] += int(k_vals[b])
    counts = [len(a) for a in assign]
    n_b = max(24, max(counts))
    n_slots = max(8, -(-max(loads) // 8) * 8)  # round up to multiple of 8

    w1 = np.concatenate(
        [W, np.ones((V, 1), np.float32)], axis=1
    ).reshape(P, IV, DD)
    w1 = np.ascontiguousarray(w1)

    in_maps = []
    for c in range(N_CORES):
        data = np.zeros((P, n_slots, IV), np.float32)
        amat = np.zeros((n_slots, n_b + 2), np.float32)
        lrefc = np.zeros((n_b, D), np.float32)
        slot = 0
        for i, b in enumerate(assign[c]):
            kb = int(k_vals[b])
            lrefc[i] = dlr[b]
            if kb:
                src = p_z[b, :kb] if use_perm[b] else mix[b, :kb]
                data[:, slot : slot + kb, :] = (
                    src.reshape(kb, P, IV).transpose(1, 0, 2)
                )
                amat[slot : slot + kb, i] = 1.0 / KMAX
                if not use_perm[b]:
                    amat[slot : slot + kb, n_b] = 1.0
                slot += kb
        amat[:, n_b + 1] = amat[:, n_b] - 1.0
        in_maps.append({"data": data, "w1": w1, "amat": amat, "lref": lrefc})
    return n_slots, n_b, in_maps, counts


def _run(inputs, trace=False, trace_cores=None):
    from concourse.bass_utils import run_bass_kernel_spmd

    n_slots, n_b, in_maps, counts = _prepare(inputs)
    key = (n_slots, n_b)
    if key not in _prog_cache:
        _prog_cache[key] = _build_program(n_slots, n_b)
    nc = _prog_cache[key]

    res = run_bass_kernel_spmd(
        nc,
        in_maps,
        list(range(N_CORES)),
        trace=trace,
        trace_cores=trace_cores,
    )
    total = 0.0
    for c in range(N_CORES):
        js = res.results[c]["jsout"][:, 0]
        total += float(js[: counts[c]].sum(dtype=np.float64))
    out = np.float32(-(0.5 * total) / 128.0)
    return out, res


def kernel(**inputs) -> np.ndarray:
    return _run(inputs)[0]
